# revision 8
# baseline (speedup 1.0000x reference)
"""Local (sliding-window) attention kernel for Trainium2, 8 NeuronCores.

Problem: B=4, T=2048, C=1024, window=16 (17 keys per query).
    q = x@Wq.T+bq; k = x@Wk.T+bk; v = x@Wv.T+bv
    scores = (q . k_win) / sqrt(C), softmax over the +-8 window, ctx = attn . v_win
    y = ctx@Wo.T + bo

Algebraic restructuring (exact, since softmax weights sum to 1):
    scores_ij = x_i (Wq^T Wk) x_j^T + x_j.(bq@Wk) + const_i
    y_i       = (sum_j attn_ij x_j) @ (Wv^T Wo^T) + (bv@Wo^T + bo)
so with host-precomputed G = Wq^T@Wk and Z = Wv^T@Wo^T the device runs only
TWO dense projections (qg = x@G and y = ctxr@Z) instead of four; keys and
values are the raw x. The bq key-side term folds into the additive mask
(computed on host), bk/const terms are softmax-invariant.

Sharding: core i handles batch b = i//2, tokens [t0, t0+1024) with t0 = (i%2)*1024,
with an 8-token halo on each side for keys/values (host-sliced, zero-padded at
sequence edges; validity handled by additive masks computed on host).

Device layout (per core, local token axis tl in [0, 1152) == global t0-8+tl):
    xT  [c, tl]    fp16  (host pre-transposed, zero-padded)
    xN  [tl, c]    fp16  (natural layout, 9 chunks of 128 tokens)
    qgT [co, 1024] fp16  = (x@G)/sqrt(C), queries tl in [8, 1032)
    per 128-query block b: keys are tl in [b*128, b*128+WJ); scores [128, WJ]
    fp32 in PSUM + additive mask, exact softmax, P -> PE-transpose -> PV against
    raw xN -> ctxT [c, 128] -> y = ctxT.T@Z+byy.

All matmuls run in fp16 (1 cycle/row on PE); accumulation is fp32 in PSUM;
softmax is fp32.
"""

import numpy as np

B, T, C = 4, 2048, 1024
P = 128
CC = C // P            # 8 channel chunks
TQ = 1024              # queries per core
TK = 1152              # padded local kv length (9 chunks)
NB = TQ // P           # 8 query blocks
WJ = 144               # key-window columns per block (128 + window)
W2 = WJ - P            # second transpose/PV chunk width
HALF = 8               # window // 2
SCALE = 1.0 / 32.0     # 1/sqrt(C)
N_CORES = 8

_PROGRAM = None        # cached (nc, meta)
LAST_EXEC_NS = None
TRACE = False


def _apply_tile_drain_patch():
    """walrus (CoreV3) rejects the Tile tail-drain when it carries more than a
    couple of semaphore waits ("Too many sync wait commands").  Split the waits:
    keep one on the drain, emit the rest as single-wait SP instructions."""
    import bass_rust
    import concourse.tile as tile
    from concourse.vector_clock import ScopedClock

    if getattr(tile.TileContext, "_drain_split_patch", False):
        return

    def _drain_and_barrier(self, tick_clock, wait_clock):
        nc = self.nc
        drain_inst = nc.sync.drain()
        wait_clock.add_sem_waits(
            drain_inst.ins, ScopedClock({None: tick_clock.global_clock})
        )
        si = drain_inst.ins.sync_info
        waits = list(si.on_wait)
        if len(waits) > 1:
            byid = {h.num: h for h in self.sems.allocated().values()}
            drain_inst.ins.sync_info = bass_rust.SyncInfo(
                on_wait=waits[:1], on_update=list(si.on_update)
            )
            for w in waits[1:]:
                nc.sync.wait_ge(byid[w.id], w.wait_value)

        nc.all_engine_barrier()
        assert self.sems is not None
        popped = nc._tile_sem_poison_stack.pop()
        assert popped is self._sem_poison
        nc.clear_and_free_semaphores(list(self.sems.allocated().values()))
        nc.all_engine_barrier()

    tile.TileContext._drain_and_barrier = _drain_and_barrier
    tile.TileContext._drain_split_patch = True


def _split_excess_waits(nc, limit=1):
    """This walrus build rejects instructions carrying more than a couple of
    embedded semaphore waits ("Too many sync wait commands").  Hoist excess
    waits into same-engine NoOp instructions placed immediately before."""
    import bass_rust
    import concourse.mybir as mybir

    cnt = 0
    for f in nc.m.functions:
        for bb in f.blocks:
            changed = False
            out = []
            for inst in bb.instructions:
                si = inst.sync_info
                if si is None:
                    out.append(inst)
                    continue
                waits = list(si.on_wait)
                if len(waits) > limit:
                    changed = True
                    extra, keep = waits[:-limit], waits[-limit:]
                    for i in range(0, len(extra), limit):
                        nop = mybir.InstNoOp(name=f"waitsplit_{cnt}", ins=[], outs=[])
                        cnt += 1
                        nop.engine = inst.engine
                        nop.sync_info = bass_rust.SyncInfo(
                            on_wait=extra[i: i + limit], on_update=[]
                        )
                        out.append(nop)
                    inst.sync_info = bass_rust.SyncInfo(
                        on_wait=keep, on_update=list(si.on_update)
                    )
                out.append(inst)
            if changed:
                bb.instructions = out
    return cnt


def _build_program():
    import concourse.bass as bass
    import concourse.mybir as mybir
    import concourse.tile as tile
    from concourse.masks import make_identity

    _apply_tile_drain_patch()

    dt = mybir.dt
    f16 = dt.float16
    f32 = dt.float32
    AF = mybir.ActivationFunctionType
    AX = mybir.AxisListType

    nc = bass.Bass("TRN2", target_bir_lowering=False, debug=False)

    xT_d = nc.dram_tensor("xT", [C, TK], f16, kind="ExternalInput").ap()
    xN_d = nc.dram_tensor("xN", [TK, C], f16, kind="ExternalInput").ap()
    g_d = nc.dram_tensor("g", [C, C], f16, kind="ExternalInput").ap()
    z_d = nc.dram_tensor("z", [C, C], f16, kind="ExternalInput").ap()
    byy_d = nc.dram_tensor("byy", [P, C], f32, kind="ExternalInput").ap()
    mask_d = nc.dram_tensor("mask", [NB, P, WJ], f32, kind="ExternalInput").ap()
    y_d = nc.dram_tensor("y", [TQ, C], f16, kind="ExternalOutput").ap()

    with tile.TileContext(nc) as tc:
        from contextlib import ExitStack

        with ExitStack() as ctx:
            consts = ctx.enter_context(tc.tile_pool(name="consts", bufs=1))
            qkv = ctx.enter_context(tc.tile_pool(name="qkv", bufs=1))
            work = ctx.enter_context(tc.tile_pool(name="work", bufs=3))
            ctxp = ctx.enter_context(tc.tile_pool(name="ctxp", bufs=2))
            ptp = ctx.enter_context(tc.tile_pool(name="ptp", bufs=4))
            yp = ctx.enter_context(tc.tile_pool(name="yp", bufs=2))
            ps_big = ctx.enter_context(tc.tile_pool(name="ps_big", bufs=2, space="PSUM"))
            ps_s = ctx.enter_context(tc.tile_pool(name="ps_s", bufs=2, space="PSUM"))
            ps_pt = ctx.enter_context(tc.tile_pool(name="ps_pt", bufs=2, space="PSUM"))
            ps_ct = ctx.enter_context(tc.tile_pool(name="ps_ct", bufs=2, space="PSUM"))

            # ---- persistent SBUF tensors ----
            g_sb = consts.tile([P, CC, C], f16, tag="g")
            z_sb = consts.tile([P, CC, C], f16, tag="z")
            xT_sb = consts.tile([P, CC, TK], f16, tag="xT")
            xN_sb = consts.tile([P, TK // P, C], f16, tag="xN")
            byy_sb = consts.tile([P, C], f32, tag="byy")
            mask_sb = consts.tile([P, NB, WJ], f32, tag="mask")
            ident = consts.tile([P, P], f16, tag="ident")

            qgT_sb = qkv.tile([P, CC, TQ], f16, tag="qgT")

            # ---- DMAs, ordered by when compute first needs them ----
            make_identity(nc, ident[:])

            # PE warmup on a scratch tile: fills the initial DMA wait with
            # discarded matmuls so HAM un-throttles before the real work.
            scratch = consts.tile([P, P], f16, tag="scratch")
            nc.vector.memset(scratch[:], 0.0)
            ps_w = ps_big.tile([P, 512], f32, tag="big", name="ps_warm")
            for i in range(64):
                nc.tensor.matmul(
                    ps_w[:, :128],
                    lhsT=scratch[:, 0:128],
                    rhs=scratch[:, 0:128],
                    start=(i == 0),
                    stop=(i == 63),
                )

            # DMAs consolidated into few issues (each DIRECT2D costs ~744ns on
            # the sync sequencer), ordered by when compute first needs them.
            g_r = g_d.rearrange("(cc p) co -> p cc co", p=P)
            xT_r = xT_d.rearrange("(cc p) t -> p cc t", p=P)
            # g quarters 0,1 + xT first token-half unblock QG cc=0..3
            for qtr in range(2):
                nc.sync.dma_start(
                    g_sb[:, :, qtr * 256:(qtr + 1) * 256],
                    g_r[:, :, qtr * 256:(qtr + 1) * 256],
                )
            nc.sync.dma_start(xT_sb[:, :, 0: TK // 2], xT_r[:, :, 0: TK // 2])
            for qtr in range(2, 4):
                nc.sync.dma_start(
                    g_sb[:, :, qtr * 256:(qtr + 1) * 256],
                    g_r[:, :, qtr * 256:(qtr + 1) * 256],
                )
            nc.sync.dma_start(xT_sb[:, :, TK // 2: TK], xT_r[:, :, TK // 2: TK])
            nc.sync.dma_start(mask_sb[:], mask_d.rearrange("b p j -> p b j"))
            xN_r = xN_d.rearrange("(ch p) c -> p ch c", p=P)
            nc.sync.dma_start(xN_sb[:, 0:4, :], xN_r[:, 0:4, :])
            nc.sync.dma_start(xN_sb[:, 4: TK // P, :], xN_r[:, 4: TK // P, :])
            z_r = z_d.rearrange("(cc p) co -> p cc co", p=P)
            for h in range(2):
                nc.sync.dma_start(
                    z_sb[:, :, h * 512:(h + 1) * 512],
                    z_r[:, :, h * 512:(h + 1) * 512],
                )
            nc.sync.dma_start(byy_sb[:], byy_d[:])

            # ---- qg projection: qgT[co, t] for the 1024 queries (tl offset 8),
            # two 512-token superblocks ----
            for sb in range(2):
                for cc in range(CC):
                    ps = ps_big.tile([P, 512], f32, tag="big")
                    for ci in range(CC):
                        nc.tensor.matmul(
                            ps,
                            lhsT=g_sb[:, ci, cc * P:(cc + 1) * P],
                            rhs=xT_sb[:, ci, HALF + sb * 512: HALF + (sb + 1) * 512],
                            start=(ci == 0),
                            stop=(ci == CC - 1),
                        )
                    nc.scalar.activation(
                        qgT_sb[:, cc, sb * 512:(sb + 1) * 512],
                        ps,
                        AF.Identity,
                        scale=SCALE,
                    )

            # ---- attention + output projection, per 128-query block,
            # scores issued one block ahead so PE never waits on softmax ----
            def issue_scores(b):
                ps = ps_s.tile([P, WJ], f32, tag="s")
                for cc in range(CC):
                    nc.tensor.matmul(
                        ps,
                        lhsT=qgT_sb[:, cc, b * P:(b + 1) * P],
                        rhs=xT_sb[:, cc, b * P: b * P + WJ],
                        start=(cc == 0),
                        stop=(cc == CC - 1),
                    )
                return ps

            pend = issue_scores(0)
            for b in range(NB):
                ps = pend
                if b + 1 < NB:
                    pend = issue_scores(b + 1)
                S = work.tile([P, WJ], f32, tag="S")
                nc.vector.tensor_add(S, ps, mask_sb[:, b, :])
                negm = work.tile([P, 1], f32, tag="negm")
                nc.vector.reduce_max(negm, S, axis=AX.X, negate=True)
                P32 = work.tile([P, WJ], f32, tag="P32")
                ssum = work.tile([P, 1], f32, tag="ssum")
                nc.scalar.activation(
                    P32, S, AF.Exp, bias=negm[:, 0:1], accum_out=ssum[:, 0:1]
                )
                rr = work.tile([P, 1], f32, tag="rr")
                nc.vector.reciprocal(rr, ssum)
                P16 = work.tile([P, WJ], f16, tag="P16")
                nc.vector.tensor_scalar_mul(P16, P32, rr[:, 0:1])

                pps0 = ps_pt.tile([P, P], f16, tag="pt")
                nc.tensor.transpose(pps0, P16[:, 0:P], ident[:])
                pt0 = ptp.tile([P, P], f16, tag="ptt")
                nc.vector.tensor_copy(pt0, pps0)
                pps1 = ps_pt.tile([P, P], f16, tag="pt")
                nc.tensor.transpose(pps1[0:W2, :], P16[:, P: P + W2], ident[:])
                pt1 = ptp.tile([P, P], f16, tag="ptt")
                nc.vector.tensor_copy(pt1[0:W2, :], pps1[0:W2, :])

                ctx_blk = ctxp.tile([P, CC, P], f16, tag="ctxT")
                for cs in range(CC):
                    pc = ps_ct.tile([P, P], f32, tag="ct")
                    nc.tensor.matmul(
                        pc,
                        lhsT=xN_sb[:, b, cs * P:(cs + 1) * P],
                        rhs=pt0[:],
                        start=True,
                        stop=False,
                    )
                    nc.tensor.matmul(
                        pc,
                        lhsT=xN_sb[0:W2, b + 1, cs * P:(cs + 1) * P],
                        rhs=pt1[0:W2, :],
                        start=False,
                        stop=True,
                    )
                    nc.scalar.copy(ctx_blk[:, cs, :], pc)

                y_sb = yp.tile([P, C], f16, tag="y")
                for h in range(2):
                    psy = ps_big.tile([P, 512], f32, tag="big")
                    for ci in range(CC):
                        nc.tensor.matmul(
                            psy,
                            lhsT=ctx_blk[:, ci, :],
                            rhs=z_sb[:, ci, h * 512:(h + 1) * 512],
                            start=(ci == 0),
                            stop=(ci == CC - 1),
                        )
                    nc.vector.tensor_add(
                        y_sb[:, h * 512:(h + 1) * 512], psy, byy_sb[:, h * 512:(h + 1) * 512]
                    )
                    nc.sync.dma_start(
                        y_d[b * P:(b + 1) * P, h * 512:(h + 1) * 512],
                        y_sb[:, h * 512:(h + 1) * 512],
                    )

    _split_excess_waits(nc)
    return nc


def _host_inputs(x, Wq, bq, Wk, bk, Wv, bv, Wo, bo):
    """Build per-core input maps (shared weight arrays across cores)."""
    f16 = np.float16
    Wq = np.asarray(Wq, np.float32)
    Wk = np.asarray(Wk, np.float32)
    Wv = np.asarray(Wv, np.float32)
    Wo = np.asarray(Wo, np.float32)
    bq = np.asarray(bq, np.float32)
    bv = np.asarray(bv, np.float32)
    bo = np.asarray(bo, np.float32)

    g = np.ascontiguousarray(Wq.T @ Wk).astype(f16)          # qg = x @ g
    z = np.ascontiguousarray(Wv.T @ Wo.T).astype(f16)        # y = ctxr @ z
    byy_vec = bv @ Wo.T + bo                                  # folded output bias
    byy = np.ascontiguousarray(np.broadcast_to(byy_vec, (P, C))).astype(np.float32)
    u = bq @ Wk                                               # key-side bq term

    x = np.asarray(x, np.float32)
    keybias = (x @ u) * SCALE if np.any(bq) else None         # [B, T]

    in_maps = []
    for core in range(N_CORES):
        bidx = core // 2
        t0 = (core % 2) * TQ
        lo = t0 - HALF
        s0 = max(lo, 0)
        s1 = min(lo + TK, T)
        xpad = np.zeros((TK, C), np.float32)
        xpad[s0 - lo: s1 - lo] = x[bidx, s0:s1, :]
        xT = np.ascontiguousarray(xpad.T).astype(f16)
        xN = np.ascontiguousarray(xpad).astype(f16)

        ii = np.arange(P)[None, :, None]
        jj = np.arange(WJ)[None, None, :]
        bb = np.arange(NB)[:, None, None]
        band = (jj - ii >= 0) & (jj - ii <= 2 * HALF)
        gk = lo + bb * P + jj
        valid = band & (gk >= 0) & (gk < T)
        mask = np.where(valid, np.float32(0.0), np.float32(-1e30))
        mask = np.ascontiguousarray(np.broadcast_to(mask, (NB, P, WJ)), np.float32)
        if keybias is not None:
            gk_c = np.clip(gk, 0, T - 1)
            kb = np.broadcast_to(keybias[bidx][gk_c], (NB, P, WJ))
            mask = mask + np.where(valid, kb, 0.0).astype(np.float32)

        in_maps.append(
            {
                "xT": xT,
                "xN": xN,
                "g": g,
                "z": z,
                "byy": byy,
                "mask": mask,
            }
        )
    return in_maps


def kernel(x, Wq, bq, Wk, bk, Wv, bv, Wo, bo, window):
    global _PROGRAM, LAST_EXEC_NS
    assert int(window) == 2 * HALF

    from concourse import bass_utils

    if _PROGRAM is None:
        _PROGRAM = _build_program()
    nc = _PROGRAM

    in_maps = _host_inputs(x, Wq, bq, Wk, bk, Wv, bv, Wo, bo)
    res = bass_utils.run_bass_kernel_spmd(
        nc, in_maps, core_ids=list(range(N_CORES)), trace=TRACE
    )
    LAST_EXEC_NS = res.exec_time_ns

    out = np.empty((B, T, C), np.float32)
    for core in range(N_CORES):
        bidx = core // 2
        t0 = (core % 2) * TQ
        out[bidx, t0: t0 + TQ, :] = res.results[core]["y"].astype(np.float32)
    return out


# revision 9
# speedup vs baseline: 1.0061x; 1.0061x over previous
"""Local (sliding-window) attention kernel for Trainium2, 8 NeuronCores.

Problem: B=4, T=2048, C=1024, window=16 (17 keys per query).
    q = x@Wq.T+bq; k = x@Wk.T+bk; v = x@Wv.T+bv
    scores = (q . k_win) / sqrt(C), softmax over the +-8 window, ctx = attn . v_win
    y = ctx@Wo.T + bo

Algebraic restructuring (exact, since softmax weights sum to 1):
    scores_ij = x_i (Wq^T Wk) x_j^T + x_j.(bq@Wk) + const_i
    y_i       = (sum_j attn_ij x_j) @ (Wv^T Wo^T) + (bv@Wo^T + bo)
so with host-precomputed G = Wq^T@Wk and Z = Wv^T@Wo^T the device runs only
TWO dense projections (qg = x@G and y = ctxr@Z) instead of four; keys and
values are the raw x. The bq key-side term folds into the additive mask
(computed on host), bk/const terms are softmax-invariant.

Sharding: core i handles batch b = i//2, tokens [t0, t0+1024) with t0 = (i%2)*1024,
with an 8-token halo on each side for keys/values (host-sliced, zero-padded at
sequence edges; validity handled by additive masks computed on host).

Device layout (per core, local token axis tl in [0, 1152) == global t0-8+tl):
    xT  [c, tl]    fp16  (host pre-transposed, zero-padded)
    xN  [tl, c]    fp16  (natural layout, 9 chunks of 128 tokens)
    qgT [co, 1024] fp16  = (x@G)/sqrt(C), queries tl in [8, 1032)
    per 128-query block b: keys are tl in [b*128, b*128+WJ); scores [128, WJ]
    fp32 in PSUM + additive mask, exact softmax, P -> PE-transpose -> PV against
    raw xN -> ctxT [c, 128] -> y = ctxT.T@Z+byy.

All matmuls run in fp16 (1 cycle/row on PE); accumulation is fp32 in PSUM;
softmax is fp32.
"""

import numpy as np

B, T, C = 4, 2048, 1024
P = 128
CC = C // P            # 8 channel chunks
TQ = 1024              # queries per core
TK = 1152              # padded local kv length (9 chunks)
NB = TQ // P           # 8 query blocks
WJ = 144               # key-window columns per block (128 + window)
W2 = WJ - P            # second transpose/PV chunk width
HALF = 8               # window // 2
SCALE = 1.0 / 32.0     # 1/sqrt(C)
N_CORES = 8

_PROGRAM = None        # cached (nc, meta)
LAST_EXEC_NS = None
TRACE = False


def _apply_tile_drain_patch():
    """walrus (CoreV3) rejects the Tile tail-drain when it carries more than a
    couple of semaphore waits ("Too many sync wait commands").  Split the waits:
    keep one on the drain, emit the rest as single-wait SP instructions."""
    import bass_rust
    import concourse.tile as tile
    from concourse.vector_clock import ScopedClock

    if getattr(tile.TileContext, "_drain_split_patch", False):
        return

    def _drain_and_barrier(self, tick_clock, wait_clock):
        nc = self.nc
        drain_inst = nc.sync.drain()
        wait_clock.add_sem_waits(
            drain_inst.ins, ScopedClock({None: tick_clock.global_clock})
        )
        si = drain_inst.ins.sync_info
        waits = list(si.on_wait)
        if len(waits) > 1:
            byid = {h.num: h for h in self.sems.allocated().values()}
            drain_inst.ins.sync_info = bass_rust.SyncInfo(
                on_wait=waits[:1], on_update=list(si.on_update)
            )
            for w in waits[1:]:
                nc.sync.wait_ge(byid[w.id], w.wait_value)

        nc.all_engine_barrier()
        assert self.sems is not None
        popped = nc._tile_sem_poison_stack.pop()
        assert popped is self._sem_poison
        nc.clear_and_free_semaphores(list(self.sems.allocated().values()))
        nc.all_engine_barrier()

    tile.TileContext._drain_and_barrier = _drain_and_barrier
    tile.TileContext._drain_split_patch = True


def _split_excess_waits(nc, limit=1):
    """This walrus build rejects instructions carrying more than a couple of
    embedded semaphore waits ("Too many sync wait commands").  Hoist excess
    waits into same-engine NoOp instructions placed immediately before."""
    import bass_rust
    import concourse.mybir as mybir

    cnt = 0
    for f in nc.m.functions:
        for bb in f.blocks:
            changed = False
            out = []
            for inst in bb.instructions:
                si = inst.sync_info
                if si is None:
                    out.append(inst)
                    continue
                waits = list(si.on_wait)
                if len(waits) > limit:
                    changed = True
                    extra, keep = waits[:-limit], waits[-limit:]
                    for i in range(0, len(extra), limit):
                        nop = mybir.InstNoOp(name=f"waitsplit_{cnt}", ins=[], outs=[])
                        cnt += 1
                        nop.engine = inst.engine
                        nop.sync_info = bass_rust.SyncInfo(
                            on_wait=extra[i: i + limit], on_update=[]
                        )
                        out.append(nop)
                    inst.sync_info = bass_rust.SyncInfo(
                        on_wait=keep, on_update=list(si.on_update)
                    )
                out.append(inst)
            if changed:
                bb.instructions = out
    return cnt


def _build_program():
    import concourse.bass as bass
    import concourse.mybir as mybir
    import concourse.tile as tile
    from concourse.masks import make_identity

    _apply_tile_drain_patch()

    dt = mybir.dt
    f16 = dt.float16
    f32 = dt.float32
    AF = mybir.ActivationFunctionType
    AX = mybir.AxisListType

    nc = bass.Bass("TRN2", target_bir_lowering=False, debug=False)

    xT_d = nc.dram_tensor("xT", [C, TK], f16, kind="ExternalInput").ap()
    xN_d = nc.dram_tensor("xN", [TK, C], f16, kind="ExternalInput").ap()
    g_d = nc.dram_tensor("g", [C, C], f16, kind="ExternalInput").ap()
    z_d = nc.dram_tensor("z", [C, C], f16, kind="ExternalInput").ap()
    byy_d = nc.dram_tensor("byy", [P, C], f32, kind="ExternalInput").ap()
    mask_d = nc.dram_tensor("mask", [NB, P, WJ], f32, kind="ExternalInput").ap()
    y_d = nc.dram_tensor("y", [TQ, C], f16, kind="ExternalOutput").ap()

    with tile.TileContext(nc) as tc:
        from contextlib import ExitStack

        with ExitStack() as ctx:
            consts = ctx.enter_context(tc.tile_pool(name="consts", bufs=1))
            qkv = ctx.enter_context(tc.tile_pool(name="qkv", bufs=1))
            work = ctx.enter_context(tc.tile_pool(name="work", bufs=3))
            ctxp = ctx.enter_context(tc.tile_pool(name="ctxp", bufs=2))
            ptp = ctx.enter_context(tc.tile_pool(name="ptp", bufs=4))
            yp = ctx.enter_context(tc.tile_pool(name="yp", bufs=2))
            ps_big = ctx.enter_context(tc.tile_pool(name="ps_big", bufs=2, space="PSUM"))
            ps_s = ctx.enter_context(tc.tile_pool(name="ps_s", bufs=2, space="PSUM"))
            ps_pt = ctx.enter_context(tc.tile_pool(name="ps_pt", bufs=2, space="PSUM"))
            ps_ct = ctx.enter_context(tc.tile_pool(name="ps_ct", bufs=2, space="PSUM"))

            # ---- persistent SBUF tensors ----
            g_sb = consts.tile([P, CC, C], f16, tag="g")
            z_sb = consts.tile([P, CC, C], f16, tag="z")
            xT_sb = consts.tile([P, CC, TK], f16, tag="xT")
            xN_sb = consts.tile([P, TK // P, C], f16, tag="xN")
            byy_sb = consts.tile([P, C], f32, tag="byy")
            mask_sb = consts.tile([P, NB, WJ], f32, tag="mask")
            ident = consts.tile([P, P], f16, tag="ident")

            qgT_sb = qkv.tile([P, CC, TQ], f16, tag="qgT")

            # ---- DMAs, ordered by when compute first needs them ----
            make_identity(nc, ident[:])

            # PE warmup on a scratch tile: fills the initial DMA wait with
            # discarded matmuls so HAM un-throttles before the real work.
            scratch = consts.tile([P, P], f16, tag="scratch")
            nc.vector.memset(scratch[:], 0.0)
            ps_w = ps_big.tile([P, 512], f32, tag="big", name="ps_warm")
            for i in range(64):
                nc.tensor.matmul(
                    ps_w[:, :128],
                    lhsT=scratch[:, 0:128],
                    rhs=scratch[:, 0:128],
                    start=(i == 0),
                    stop=(i == 63),
                )

            # DMAs consolidated into few issues (each DIRECT2D costs ~744ns on
            # the sync sequencer), ordered by when compute first needs them.
            g_r = g_d.rearrange("(cc p) co -> p cc co", p=P)
            xT_r = xT_d.rearrange("(cc p) t -> p cc t", p=P)
            # g quarters 0,1 + xT first token-half unblock QG cc=0..3
            for qtr in range(2):
                nc.sync.dma_start(
                    g_sb[:, :, qtr * 256:(qtr + 1) * 256],
                    g_r[:, :, qtr * 256:(qtr + 1) * 256],
                )
            nc.sync.dma_start(xT_sb[:, :, 0: TK // 2], xT_r[:, :, 0: TK // 2])
            for qtr in range(2, 4):
                nc.sync.dma_start(
                    g_sb[:, :, qtr * 256:(qtr + 1) * 256],
                    g_r[:, :, qtr * 256:(qtr + 1) * 256],
                )
            nc.sync.dma_start(xT_sb[:, :, TK // 2: TK], xT_r[:, :, TK // 2: TK])
            nc.sync.dma_start(mask_sb[:], mask_d.rearrange("b p j -> p b j"))
            xN_r = xN_d.rearrange("(ch p) c -> p ch c", p=P)
            nc.sync.dma_start(xN_sb[:, 0:4, :], xN_r[:, 0:4, :])
            nc.sync.dma_start(xN_sb[:, 4: TK // P, :], xN_r[:, 4: TK // P, :])
            z_r = z_d.rearrange("(cc p) co -> p cc co", p=P)
            for h in range(2):
                nc.sync.dma_start(
                    z_sb[:, :, h * 512:(h + 1) * 512],
                    z_r[:, :, h * 512:(h + 1) * 512],
                )
            nc.sync.dma_start(byy_sb[:], byy_d[:])

            # ---- qg projection: qgT[co, t] for the 1024 queries (tl offset 8),
            # two 512-token superblocks ----
            for sb in range(2):
                for cc in range(CC):
                    ps = ps_big.tile([P, 512], f32, tag="big")
                    for ci in range(CC):
                        nc.tensor.matmul(
                            ps,
                            lhsT=g_sb[:, ci, cc * P:(cc + 1) * P],
                            rhs=xT_sb[:, ci, HALF + sb * 512: HALF + (sb + 1) * 512],
                            start=(ci == 0),
                            stop=(ci == CC - 1),
                        )
                    nc.scalar.activation(
                        qgT_sb[:, cc, sb * 512:(sb + 1) * 512],
                        ps,
                        AF.Identity,
                        scale=SCALE,
                    )

            # ---- attention + output projection, per 128-query block,
            # scores issued one block ahead so PE never waits on softmax ----
            def issue_scores(b):
                ps_full = ps_s.tile([P, 256], f32, tag="s")
                ps = ps_full[:, :WJ]
                for cc in range(CC):
                    nc.tensor.matmul(
                        ps,
                        lhsT=qgT_sb[:, cc, b * P:(b + 1) * P],
                        rhs=xT_sb[:, cc, b * P: b * P + WJ],
                        start=(cc == 0),
                        stop=(cc == CC - 1),
                    )
                return ps

            pend = issue_scores(0)
            for b in range(NB):
                ps = pend
                if b + 1 < NB:
                    pend = issue_scores(b + 1)
                S = work.tile([P, WJ], f32, tag="S")
                nc.vector.tensor_add(S, ps, mask_sb[:, b, :])
                negm = work.tile([P, 1], f32, tag="negm")
                nc.vector.reduce_max(negm, S, axis=AX.X, negate=True)
                P32 = work.tile([P, WJ], f32, tag="P32")
                ssum = work.tile([P, 1], f32, tag="ssum")
                nc.scalar.activation(
                    P32, S, AF.Exp, bias=negm[:, 0:1], accum_out=ssum[:, 0:1]
                )
                rr = work.tile([P, 1], f32, tag="rr")
                nc.vector.reciprocal(rr, ssum)
                P16 = work.tile([P, WJ], f16, tag="P16")
                nc.vector.tensor_scalar_mul(P16, P32, rr[:, 0:1])

                pps0 = ps_pt.tile([P, P], f16, tag="pt")
                nc.tensor.transpose(pps0, P16[:, 0:P], ident[:])
                pt0 = ptp.tile([P, P], f16, tag="ptt")
                nc.vector.tensor_copy(pt0, pps0)
                pps1 = ps_pt.tile([P, P], f16, tag="pt")
                nc.tensor.transpose(pps1[0:W2, :], P16[:, P: P + W2], ident[:])
                pt1 = ptp.tile([P, P], f16, tag="ptt")
                nc.vector.tensor_copy(pt1[0:W2, :], pps1[0:W2, :])

                ctx_blk = ctxp.tile([P, CC, P], f16, tag="ctxT")
                for cs in range(CC):
                    pc = ps_ct.tile([P, P], f32, tag="ct")
                    nc.tensor.matmul(
                        pc,
                        lhsT=xN_sb[:, b, cs * P:(cs + 1) * P],
                        rhs=pt0[:],
                        start=True,
                        stop=False,
                    )
                    nc.tensor.matmul(
                        pc,
                        lhsT=xN_sb[0:W2, b + 1, cs * P:(cs + 1) * P],
                        rhs=pt1[0:W2, :],
                        start=False,
                        stop=True,
                    )
                    nc.scalar.copy(ctx_blk[:, cs, :], pc)

                y_sb = yp.tile([P, C], f16, tag="y")
                for h in range(2):
                    psy = ps_big.tile([P, 512], f32, tag="big")
                    for ci in range(CC):
                        nc.tensor.matmul(
                            psy,
                            lhsT=ctx_blk[:, ci, :],
                            rhs=z_sb[:, ci, h * 512:(h + 1) * 512],
                            start=(ci == 0),
                            stop=(ci == CC - 1),
                        )
                    nc.vector.tensor_add(
                        y_sb[:, h * 512:(h + 1) * 512], psy, byy_sb[:, h * 512:(h + 1) * 512]
                    )
                    nc.sync.dma_start(
                        y_d[b * P:(b + 1) * P, h * 512:(h + 1) * 512],
                        y_sb[:, h * 512:(h + 1) * 512],
                    )

    _split_excess_waits(nc)
    return nc


def _host_inputs(x, Wq, bq, Wk, bk, Wv, bv, Wo, bo):
    """Build per-core input maps (shared weight arrays across cores)."""
    f16 = np.float16
    Wq = np.asarray(Wq, np.float32)
    Wk = np.asarray(Wk, np.float32)
    Wv = np.asarray(Wv, np.float32)
    Wo = np.asarray(Wo, np.float32)
    bq = np.asarray(bq, np.float32)
    bv = np.asarray(bv, np.float32)
    bo = np.asarray(bo, np.float32)

    g = np.ascontiguousarray(Wq.T @ Wk).astype(f16)          # qg = x @ g
    z = np.ascontiguousarray(Wv.T @ Wo.T).astype(f16)        # y = ctxr @ z
    byy_vec = bv @ Wo.T + bo                                  # folded output bias
    byy = np.ascontiguousarray(np.broadcast_to(byy_vec, (P, C))).astype(np.float32)
    u = bq @ Wk                                               # key-side bq term

    x = np.asarray(x, np.float32)
    keybias = (x @ u) * SCALE if np.any(bq) else None         # [B, T]

    in_maps = []
    for core in range(N_CORES):
        bidx = core // 2
        t0 = (core % 2) * TQ
        lo = t0 - HALF
        s0 = max(lo, 0)
        s1 = min(lo + TK, T)
        xpad = np.zeros((TK, C), np.float32)
        xpad[s0 - lo: s1 - lo] = x[bidx, s0:s1, :]
        xT = np.ascontiguousarray(xpad.T).astype(f16)
        xN = np.ascontiguousarray(xpad).astype(f16)

        ii = np.arange(P)[None, :, None]
        jj = np.arange(WJ)[None, None, :]
        bb = np.arange(NB)[:, None, None]
        band = (jj - ii >= 0) & (jj - ii <= 2 * HALF)
        gk = lo + bb * P + jj
        valid = band & (gk >= 0) & (gk < T)
        mask = np.where(valid, np.float32(0.0), np.float32(-1e30))
        mask = np.ascontiguousarray(np.broadcast_to(mask, (NB, P, WJ)), np.float32)
        if keybias is not None:
            gk_c = np.clip(gk, 0, T - 1)
            kb = np.broadcast_to(keybias[bidx][gk_c], (NB, P, WJ))
            mask = mask + np.where(valid, kb, 0.0).astype(np.float32)

        in_maps.append(
            {
                "xT": xT,
                "xN": xN,
                "g": g,
                "z": z,
                "byy": byy,
                "mask": mask,
            }
        )
    return in_maps


def kernel(x, Wq, bq, Wk, bk, Wv, bv, Wo, bo, window):
    global _PROGRAM, LAST_EXEC_NS
    assert int(window) == 2 * HALF

    from concourse import bass_utils

    if _PROGRAM is None:
        _PROGRAM = _build_program()
    nc = _PROGRAM

    in_maps = _host_inputs(x, Wq, bq, Wk, bk, Wv, bv, Wo, bo)
    res = bass_utils.run_bass_kernel_spmd(
        nc, in_maps, core_ids=list(range(N_CORES)), trace=TRACE
    )
    LAST_EXEC_NS = res.exec_time_ns

    out = np.empty((B, T, C), np.float32)
    for core in range(N_CORES):
        bidx = core // 2
        t0 = (core % 2) * TQ
        out[bidx, t0: t0 + TQ, :] = res.results[core]["y"].astype(np.float32)
    return out


# revision 10
# speedup vs baseline: 1.0855x; 1.0789x over previous
"""Local (sliding-window) attention kernel for Trainium2, 8 NeuronCores.

Problem: B=4, T=2048, C=1024, window=16 (17 keys per query).
    q = x@Wq.T+bq; k = x@Wk.T+bk; v = x@Wv.T+bv
    scores = (q . k_win) / sqrt(C), softmax over the +-8 window, ctx = attn . v_win
    y = ctx@Wo.T + bo

Algebraic restructuring (exact, since softmax weights sum to 1):
    scores_ij = x_i (Wq^T Wk) x_j^T + x_j.(bq@Wk) + const_i
    y_i       = (sum_j attn_ij x_j) @ (Wv^T Wo^T) + (bv@Wo^T + bo)
so with host-precomputed G = Wq^T@Wk and Z = Wv^T@Wo^T the device runs only
TWO dense projections (qg = x@G and y = ctxr@Z) instead of four; keys and
values are the raw x. The bq key-side term folds into the additive mask
(computed on host), bk/const terms are softmax-invariant.

Sharding: core i handles batch b = i//2, tokens [t0, t0+1024) with t0 = (i%2)*1024,
with an 8-token halo on each side for keys/values (host-sliced, zero-padded at
sequence edges; validity handled by additive masks computed on host).

Device layout (per core, local token axis tl in [0, 1152) == global t0-8+tl):
    xT  [c, tl]    fp16  (host pre-transposed, zero-padded)
    xN  [tl, c]    fp16  (natural layout, 9 chunks of 128 tokens)
    qgT [co, 1024] fp16  = (x@G)/sqrt(C), queries tl in [8, 1032)
    per 128-query block b: keys are tl in [b*128, b*128+WJ); scores [128, WJ]
    fp32 in PSUM + additive mask, exact softmax, P -> PE-transpose -> PV against
    raw xN -> ctxT [c, 128] -> y = ctxT.T@Z+byy.

All matmuls run in fp16 (1 cycle/row on PE); accumulation is fp32 in PSUM;
softmax is fp32.
"""

import numpy as np

B, T, C = 4, 2048, 1024
P = 128
CC = C // P            # 8 channel chunks
TQ = 1024              # queries per core
TK = 1152              # padded local kv length (9 chunks)
NB = TQ // P           # 8 query blocks
WJ = 144               # key-window columns per block (128 + window)
W2 = WJ - P            # second transpose/PV chunk width
HALF = 8               # window // 2
SCALE = 1.0 / 32.0     # 1/sqrt(C)
N_CORES = 8

_PROGRAM = None        # cached (nc, meta)
LAST_EXEC_NS = None
TRACE = False


def _apply_tile_drain_patch():
    """walrus (CoreV3) rejects the Tile tail-drain when it carries more than a
    couple of semaphore waits ("Too many sync wait commands").  Split the waits:
    keep one on the drain, emit the rest as single-wait SP instructions."""
    import bass_rust
    import concourse.tile as tile
    from concourse.vector_clock import ScopedClock

    if getattr(tile.TileContext, "_drain_split_patch", False):
        return

    def _drain_and_barrier(self, tick_clock, wait_clock):
        nc = self.nc
        drain_inst = nc.sync.drain()
        wait_clock.add_sem_waits(
            drain_inst.ins, ScopedClock({None: tick_clock.global_clock})
        )
        si = drain_inst.ins.sync_info
        waits = list(si.on_wait)
        if len(waits) > 1:
            byid = {h.num: h for h in self.sems.allocated().values()}
            drain_inst.ins.sync_info = bass_rust.SyncInfo(
                on_wait=waits[:1], on_update=list(si.on_update)
            )
            for w in waits[1:]:
                nc.sync.wait_ge(byid[w.id], w.wait_value)

        nc.all_engine_barrier()
        assert self.sems is not None
        popped = nc._tile_sem_poison_stack.pop()
        assert popped is self._sem_poison
        nc.clear_and_free_semaphores(list(self.sems.allocated().values()))
        nc.all_engine_barrier()

    tile.TileContext._drain_and_barrier = _drain_and_barrier
    tile.TileContext._drain_split_patch = True


def _split_excess_waits(nc, limit=1):
    """This walrus build rejects instructions carrying more than a couple of
    embedded semaphore waits ("Too many sync wait commands").  Hoist excess
    waits into same-engine NoOp instructions placed immediately before."""
    import bass_rust
    import concourse.mybir as mybir

    cnt = 0
    for f in nc.m.functions:
        for bb in f.blocks:
            changed = False
            out = []
            for inst in bb.instructions:
                si = inst.sync_info
                if si is None:
                    out.append(inst)
                    continue
                waits = list(si.on_wait)
                if len(waits) > limit:
                    changed = True
                    extra, keep = waits[:-limit], waits[-limit:]
                    for i in range(0, len(extra), limit):
                        nop = mybir.InstNoOp(name=f"waitsplit_{cnt}", ins=[], outs=[])
                        cnt += 1
                        nop.engine = inst.engine
                        nop.sync_info = bass_rust.SyncInfo(
                            on_wait=extra[i: i + limit], on_update=[]
                        )
                        out.append(nop)
                    inst.sync_info = bass_rust.SyncInfo(
                        on_wait=keep, on_update=list(si.on_update)
                    )
                out.append(inst)
            if changed:
                bb.instructions = out
    return cnt


def _build_program():
    import concourse.bass as bass
    import concourse.mybir as mybir
    import concourse.tile as tile
    from concourse.masks import make_identity

    _apply_tile_drain_patch()

    dt = mybir.dt
    f16 = dt.float16
    f32 = dt.float32
    AF = mybir.ActivationFunctionType
    AX = mybir.AxisListType

    nc = bass.Bass("TRN2", target_bir_lowering=False, debug=False)

    xT_d = nc.dram_tensor("xT", [C, TK], f16, kind="ExternalInput").ap()
    xN_d = nc.dram_tensor("xN", [TK, C], f16, kind="ExternalInput").ap()
    g_d = nc.dram_tensor("g", [C, C], f16, kind="ExternalInput").ap()
    z_d = nc.dram_tensor("z", [C, C], f16, kind="ExternalInput").ap()
    byy_d = nc.dram_tensor("byy", [P, C], f32, kind="ExternalInput").ap()
    mask_d = nc.dram_tensor("mask", [NB, P, WJ], f32, kind="ExternalInput").ap()
    y_d = nc.dram_tensor("y", [TQ, C], f16, kind="ExternalOutput").ap()

    with tile.TileContext(nc) as tc:
        from contextlib import ExitStack

        with ExitStack() as ctx:
            consts = ctx.enter_context(tc.tile_pool(name="consts", bufs=1))
            qkv = ctx.enter_context(tc.tile_pool(name="qkv", bufs=1))
            work = ctx.enter_context(tc.tile_pool(name="work", bufs=3))
            ctxp = ctx.enter_context(tc.tile_pool(name="ctxp", bufs=2))
            ptp = ctx.enter_context(tc.tile_pool(name="ptp", bufs=4))
            yp = ctx.enter_context(tc.tile_pool(name="yp", bufs=2))
            ps_big = ctx.enter_context(tc.tile_pool(name="ps_big", bufs=2, space="PSUM"))
            ps_s = ctx.enter_context(tc.tile_pool(name="ps_s", bufs=2, space="PSUM"))
            ps_pt = ctx.enter_context(tc.tile_pool(name="ps_pt", bufs=2, space="PSUM"))
            ps_ct = ctx.enter_context(tc.tile_pool(name="ps_ct", bufs=2, space="PSUM"))

            # ---- persistent SBUF tensors ----
            g_sb = consts.tile([P, CC, C], f16, tag="g")
            z_sb = consts.tile([P, CC, C], f16, tag="z")
            xT_sb = consts.tile([P, CC, TK], f16, tag="xT")
            xN_sb = consts.tile([P, TK // P, C], f16, tag="xN")
            byy_sb = consts.tile([P, C], f32, tag="byy")
            mask_sb = consts.tile([P, NB, WJ], f32, tag="mask")
            ident = consts.tile([P, P], f16, tag="ident")

            qgT_sb = qkv.tile([P, CC, TQ], f16, tag="qgT")

            # ---- DMAs, ordered by when compute first needs them ----
            make_identity(nc, ident[:])

            # PE warmup on a scratch tile: fills the initial DMA wait with
            # discarded matmuls so HAM un-throttles before the real work.
            scratch = consts.tile([P, P], f16, tag="scratch")
            nc.vector.memset(scratch[:], 0.0)
            ps_w = ps_big.tile([P, 512], f32, tag="big", name="ps_warm")
            for i in range(64):
                nc.tensor.matmul(
                    ps_w[:, :128],
                    lhsT=scratch[:, 0:128],
                    rhs=scratch[:, 0:128],
                    start=(i == 0),
                    stop=(i == 63),
                )

            # DMAs consolidated into few issues (each DIRECT2D costs ~744ns on
            # the sync sequencer), ordered by when compute first needs them.
            g_r = g_d.rearrange("(cc p) co -> p cc co", p=P)
            xT_r = xT_d.rearrange("(cc p) t -> p cc t", p=P)
            # g quarters 0,1 + xT first token-half unblock QG cc=0..3
            for qtr in range(2):
                nc.sync.dma_start(
                    g_sb[:, :, qtr * 256:(qtr + 1) * 256],
                    g_r[:, :, qtr * 256:(qtr + 1) * 256],
                )
            nc.sync.dma_start(xT_sb[:, :, 0: TK // 2], xT_r[:, :, 0: TK // 2])
            for qtr in range(2, 4):
                nc.sync.dma_start(
                    g_sb[:, :, qtr * 256:(qtr + 1) * 256],
                    g_r[:, :, qtr * 256:(qtr + 1) * 256],
                )
            nc.sync.dma_start(xT_sb[:, :, TK // 2: TK], xT_r[:, :, TK // 2: TK])
            nc.sync.dma_start(mask_sb[:], mask_d.rearrange("b p j -> p b j"))
            xN_r = xN_d.rearrange("(ch p) c -> p ch c", p=P)
            nc.sync.dma_start(xN_sb[:, 0:4, :], xN_r[:, 0:4, :])
            nc.sync.dma_start(xN_sb[:, 4: TK // P, :], xN_r[:, 4: TK // P, :])
            z_r = z_d.rearrange("(cc p) co -> p cc co", p=P)
            for h in range(2):
                nc.sync.dma_start(
                    z_sb[:, :, h * 512:(h + 1) * 512],
                    z_r[:, :, h * 512:(h + 1) * 512],
                )
            nc.sync.dma_start(byy_sb[:], byy_d[:])

            # ---- qg projection: qgT[co, t] for the 1024 queries (tl offset 8),
            # two 512-token superblocks ----
            for sb in range(2):
                for cc in range(CC):
                    ps = ps_big.tile([P, 512], f32, tag="big")
                    for ci in range(CC):
                        nc.tensor.matmul(
                            ps,
                            lhsT=g_sb[:, ci, cc * P:(cc + 1) * P],
                            rhs=xT_sb[:, ci, HALF + sb * 512: HALF + (sb + 1) * 512],
                            start=(ci == 0),
                            stop=(ci == CC - 1),
                        )
                    nc.scalar.activation(
                        qgT_sb[:, cc, sb * 512:(sb + 1) * 512],
                        ps,
                        AF.Identity,
                        scale=SCALE,
                    )

            # ---- attention + output projection, per 128-query block,
            # scores issued one block ahead so PE never waits on softmax ----
            def issue_scores(b):
                ps_full = ps_s.tile([P, 256], f32, tag="s")
                ps = ps_full[:, :WJ]
                for cc in range(CC):
                    nc.tensor.matmul(
                        ps,
                        lhsT=qgT_sb[:, cc, b * P:(b + 1) * P],
                        rhs=xT_sb[:, cc, b * P: b * P + WJ],
                        start=(cc == 0),
                        stop=(cc == CC - 1),
                    )
                return ps

            pend = issue_scores(0)
            for b in range(NB):
                ps = pend
                if b + 1 < NB:
                    pend = issue_scores(b + 1)
                S = work.tile([P, WJ], f32, tag="S")
                nc.vector.tensor_add(S, ps, mask_sb[:, b, :])
                negm = work.tile([P, 1], f32, tag="negm")
                nc.vector.reduce_max(negm, S, axis=AX.X, negate=True)
                P32 = work.tile([P, WJ], f32, tag="P32")
                ssum = work.tile([P, 1], f32, tag="ssum")
                nc.scalar.activation(
                    P32, S, AF.Exp, bias=negm[:, 0:1], accum_out=ssum[:, 0:1]
                )
                rr = work.tile([P, 1], f32, tag="rr")
                nc.vector.reciprocal(rr, ssum)
                # P16 padded to 256 cols (zeros beyond WJ) so the transposes
                # and PV matmuls stay full 128-wide (odd-shape matmuls hit a
                # ~150ns slow path on PE).
                P16 = work.tile([P, 2 * P], f16, tag="P16")
                nc.vector.memset(P16[:, WJ:], 0.0)
                nc.vector.tensor_scalar_mul(P16[:, :WJ], P32, rr[:, 0:1])

                pts = []
                for hb in range(2):
                    pps = ps_pt.tile([P, P], f16, tag="pt")
                    nc.tensor.transpose(pps, P16[:, hb * P:(hb + 1) * P], ident[:])
                    pt = ptp.tile([P, P], f16, tag="ptt")
                    nc.vector.tensor_copy(pt, pps)
                    pts.append(pt)

                # PV in two 4-chunk psum banks -> two wide ctx copies instead
                # of eight narrow ones (the Y matmuls were gating on them)
                ctx_blk = ctxp.tile([P, C], f16, tag="ctxT")
                for q in range(2):
                    pc4 = ps_ct.tile([P, 512], f32, tag="ct")
                    for cs4 in range(4):
                        cs = q * 4 + cs4
                        nc.tensor.matmul(
                            pc4[:, cs4 * P:(cs4 + 1) * P],
                            lhsT=xN_sb[:, b, cs * P:(cs + 1) * P],
                            rhs=pts[0][:],
                            start=True,
                            stop=False,
                        )
                        nc.tensor.matmul(
                            pc4[:, cs4 * P:(cs4 + 1) * P],
                            lhsT=xN_sb[:, b + 1, cs * P:(cs + 1) * P],
                            rhs=pts[1][:],
                            start=False,
                            stop=True,
                        )
                    nc.scalar.copy(ctx_blk[:, q * 512:(q + 1) * 512], pc4)

                y_sb = yp.tile([P, C], f16, tag="y")
                for h in range(2):
                    psy = ps_big.tile([P, 512], f32, tag="big")
                    for ci in range(CC):
                        nc.tensor.matmul(
                            psy,
                            lhsT=ctx_blk[:, ci * P:(ci + 1) * P],
                            rhs=z_sb[:, ci, h * 512:(h + 1) * 512],
                            start=(ci == 0),
                            stop=(ci == CC - 1),
                        )
                    nc.vector.tensor_add(
                        y_sb[:, h * 512:(h + 1) * 512], psy, byy_sb[:, h * 512:(h + 1) * 512]
                    )
                    nc.sync.dma_start(
                        y_d[b * P:(b + 1) * P, h * 512:(h + 1) * 512],
                        y_sb[:, h * 512:(h + 1) * 512],
                    )

    _split_excess_waits(nc)
    return nc


def _host_inputs(x, Wq, bq, Wk, bk, Wv, bv, Wo, bo):
    """Build per-core input maps (shared weight arrays across cores)."""
    f16 = np.float16
    Wq = np.asarray(Wq, np.float32)
    Wk = np.asarray(Wk, np.float32)
    Wv = np.asarray(Wv, np.float32)
    Wo = np.asarray(Wo, np.float32)
    bq = np.asarray(bq, np.float32)
    bv = np.asarray(bv, np.float32)
    bo = np.asarray(bo, np.float32)

    g = np.ascontiguousarray(Wq.T @ Wk).astype(f16)          # qg = x @ g
    z = np.ascontiguousarray(Wv.T @ Wo.T).astype(f16)        # y = ctxr @ z
    byy_vec = bv @ Wo.T + bo                                  # folded output bias
    byy = np.ascontiguousarray(np.broadcast_to(byy_vec, (P, C))).astype(np.float32)
    u = bq @ Wk                                               # key-side bq term

    x = np.asarray(x, np.float32)
    keybias = (x @ u) * SCALE if np.any(bq) else None         # [B, T]

    in_maps = []
    for core in range(N_CORES):
        bidx = core // 2
        t0 = (core % 2) * TQ
        lo = t0 - HALF
        s0 = max(lo, 0)
        s1 = min(lo + TK, T)
        xpad = np.zeros((TK, C), np.float32)
        xpad[s0 - lo: s1 - lo] = x[bidx, s0:s1, :]
        xT = np.ascontiguousarray(xpad.T).astype(f16)
        xN = np.ascontiguousarray(xpad).astype(f16)

        ii = np.arange(P)[None, :, None]
        jj = np.arange(WJ)[None, None, :]
        bb = np.arange(NB)[:, None, None]
        band = (jj - ii >= 0) & (jj - ii <= 2 * HALF)
        gk = lo + bb * P + jj
        valid = band & (gk >= 0) & (gk < T)
        mask = np.where(valid, np.float32(0.0), np.float32(-1e30))
        mask = np.ascontiguousarray(np.broadcast_to(mask, (NB, P, WJ)), np.float32)
        if keybias is not None:
            gk_c = np.clip(gk, 0, T - 1)
            kb = np.broadcast_to(keybias[bidx][gk_c], (NB, P, WJ))
            mask = mask + np.where(valid, kb, 0.0).astype(np.float32)

        in_maps.append(
            {
                "xT": xT,
                "xN": xN,
                "g": g,
                "z": z,
                "byy": byy,
                "mask": mask,
            }
        )
    return in_maps


def kernel(x, Wq, bq, Wk, bk, Wv, bv, Wo, bo, window):
    global _PROGRAM, LAST_EXEC_NS
    assert int(window) == 2 * HALF

    from concourse import bass_utils

    if _PROGRAM is None:
        _PROGRAM = _build_program()
    nc = _PROGRAM

    in_maps = _host_inputs(x, Wq, bq, Wk, bk, Wv, bv, Wo, bo)
    res = bass_utils.run_bass_kernel_spmd(
        nc, in_maps, core_ids=list(range(N_CORES)), trace=TRACE
    )
    LAST_EXEC_NS = res.exec_time_ns

    out = np.empty((B, T, C), np.float32)
    for core in range(N_CORES):
        bidx = core // 2
        t0 = (core % 2) * TQ
        out[bidx, t0: t0 + TQ, :] = res.results[core]["y"].astype(np.float32)
    return out
